# revision 1
# baseline (speedup 1.0000x reference)
"""Paged-attention GQA decode kernel for Trainium2 (8 NeuronCores).

Problem: B=32 sequences, one new token each; KV cache [65536, 8, 128] f32
paged with PAGE=16; 32 query heads, 8 KV heads (GQA group 4), D=128.

Strategy (tensor-parallel over KV heads, one head per core):
  host:  scatter new k/v into per-head packed K^T / V tile layouts,
         valid-length (128-padded) only; block-diagonal sorted layout.
  device (per core, one KV head):
     mm1:  scores[bg, j] = qT_blk.T @ K^T  -- block-diagonal accumulation,
           M=32 partition blocks (8 seqs each), PSUM pending-zero semantics
     softmax: exp (ACT, per block), mask multiply, row-sum, reciprocal
              (normalization deferred to host)
     mm2:  oT[d, bg] += V_chunk.T @ pT_chunk  -- V-stationary, PSUM accum
  host:  o[b, h] = oT[:, col] * r[col], un-permute, concat heads.

The device program is value-specialized on the per-seq chunk counts
(derived from context_lens); compiled programs are cached per signature.
"""

import numpy as np

B = 32
KV_LEN = 2048
PAGE = 16
PAGES = KV_LEN // PAGE
H_Q = 32
H_KV = 8
D = 128
G = H_Q // H_KV          # 4
CH = 128                 # slot chunk (matmul contraction tile)
NCORES = 8
P = 128
SCALE = np.float32(1.0 / np.sqrt(D))

_PROGRAM_CACHE = {}


def _rowbase(s):
    return 32 * (s // 8) + 4 * (s % 8)


def _build_program(nt_s):
    """Build + compile the per-core Bass program, specialized on the sorted
    per-seq chunk counts nt_s (descending)."""
    import concourse.bacc as bacc
    import concourse.mybir as mybir
    import concourse.tile as tile
    from concourse.masks import make_identity
    from concourse.tile import add_dep_helper

    f32 = mybir.dt.float32
    vp_s = [n * CH for n in nt_s]
    k_off = np.zeros(B + 1, np.int64)
    k_off[1:] = np.cumsum(vp_s)
    TOTK = int(k_off[-1])
    maxpad = vp_s[0]
    maxnt = nt_s[0]
    bm = [vp_s[k * 8] for k in range(4)]  # per-32-block max width

    nc = bacc.Bacc(
        "TRN2",
        target_bir_lowering=False,
        debug=False,
        enable_asserts=False,
        num_devices=NCORES,
    )
    qT_d = nc.dram_tensor("qT", [P, B * 32], f32, kind="ExternalInput").ap()
    kT_d = nc.dram_tensor("kT", [P, TOTK], f32, kind="ExternalInput").ap()
    vt_d = nc.dram_tensor("vt", [P, TOTK], f32, kind="ExternalInput").ap()
    mask_d = nc.dram_tensor("mask", [P, maxpad], f32, kind="ExternalInput").ap()
    oT_d = nc.dram_tensor("oT", [P, P], f32, kind="ExternalOutput").ap()
    r_d = nc.dram_tensor("r", [P, 1], f32, kind="ExternalOutput").ap()

    with tile.TileContext(nc) as tc:
        with (
            tc.tile_pool(name="const", bufs=1) as constp,
            tc.tile_pool(name="kpool", bufs=4) as kpool,
            tc.tile_pool(name="vpool", bufs=4) as vpool,
            tc.tile_pool(name="sm", bufs=1) as smp,
            tc.tile_pool(name="tp", bufs=2, space="PSUM") as tpp,
            tc.tile_pool(name="ps_scores", bufs=1, space="PSUM") as pssc,
            tc.tile_pool(name="ps_o", bufs=1, space="PSUM") as pso,
        ):
            qT_s = constp.tile([P, B * 32], f32)
            nc.sync.dma_start(qT_s[:], qT_d[:])
            mask_s = constp.tile([P, maxpad], f32)
            nc.sync.dma_start(mask_s[:], mask_d[:])
            ident = constp.tile([P, P], f32)
            make_identity(nc, ident[:])

            e_t = smp.tile([P, maxpad], f32)
            nc.gpsimd.memset(e_t[:], 0.0)

            scores_ps = pssc.tile([P, maxpad], f32, space="PSUM")

            # ---- phase 1: K loads + block-diagonal mm1 ----
            first_mm = {}
            for s in range(B):
                vps = vp_s[s]
                blk = s // 8
                kt = kpool.tile([P, maxpad], f32, tag="k")
                nc.sync.dma_start(kt[:, :vps],
                                  kT_d[:, int(k_off[s]):int(k_off[s + 1])])
                for r0 in range(0, vps, 512):
                    n = min(512, vps - r0)
                    bank = r0 // 512
                    is_first = s % 8 == 0
                    mm = nc.tensor.matmul(
                        scores_ps[32 * blk:32 * blk + 32, r0:r0 + n],
                        qT_s[:, 32 * s:32 * s + 32],
                        kt[:, r0:r0 + n],
                        start=is_first, stop=False,
                        skip_group_check=True,
                        tile_position=(0, 32 * blk),
                    )
                    if is_first:
                        first_mm.setdefault(blk, {})[bank] = mm
                    else:
                        add_dep_helper(
                            mm.ins, first_mm[blk][bank].ins,
                            reason="pending-zero: block-first mm1 first")

            # ---- softmax (deferred normalization) ----
            for k in range(4):
                nc.scalar.activation(
                    e_t[32 * k:32 * k + 32, :bm[k]],
                    scores_ps[32 * k:32 * k + 32, :bm[k]],
                    mybir.ActivationFunctionType.Exp)
            em_t = smp.tile([P, maxpad], f32)
            nc.vector.tensor_mul(em_t[:], e_t[:], mask_s[:])
            s_t = smp.tile([P, 1], f32)
            nc.vector.reduce_sum(s_t[:], em_t[:], axis=mybir.AxisListType.X)
            r_t = smp.tile([P, 1], f32)
            nc.vector.reciprocal(r_t[:], s_t[:])
            nc.sync.dma_start(r_d[:], r_t[:])

            # ---- transposes of masked-exp chunks ----
            pT = smp.tile([P, maxpad], f32)
            for c in range(maxnt):
                tp = tpp.tile([P, P], f32, space="PSUM", tag="tp")
                nc.tensor.transpose(tp[:], em_t[:, c * CH:(c + 1) * CH], ident[:])
                nc.vector.tensor_copy(pT[:, c * CH:(c + 1) * CH], tp[:])

            # ---- phase 2: V loads + V-stationary mm2 ----
            oT_ps = pso.tile([P, P], f32, space="PSUM")
            first_mm2 = None
            for s in range(B):
                nts = nt_s[s]
                rb = _rowbase(s)
                vt = vpool.tile([P, maxpad], f32, tag="v")
                nc.sync.dma_start(vt[:, :nts * D],
                                  vt_d[:, int(k_off[s]):int(k_off[s + 1])])
                for c in range(nts):
                    mm = nc.tensor.matmul(
                        oT_ps[:, rb:rb + 4],
                        vt[:, c * D:(c + 1) * D],
                        pT[:, c * CH + rb:c * CH + rb + 4],
                        start=(first_mm2 is None), stop=False,
                        skip_group_check=True,
                    )
                    if first_mm2 is None:
                        first_mm2 = mm
                    elif c == 0:
                        add_dep_helper(
                            mm.ins, first_mm2.ins,
                            reason="pending-zero: global-first mm2 first")

            oT_sb = smp.tile([P, P], f32)
            nc.vector.tensor_copy(oT_sb[:], oT_ps[:])
            nc.sync.dma_start(oT_d[:], oT_sb[:])

    nc.compile()
    return nc


def _host_prep(q, k, v, k_cache, v_cache, slot_mapping, block_tables,
               context_lens):
    """Build per-core packed inputs. Returns (in_maps, perm, meta)."""
    ctx = np.clip(np.asarray(context_lens, np.int64), 1, KV_LEN)
    nt = (ctx + CH - 1) // CH
    vp = nt * CH
    perm = np.argsort(-vp, kind="stable")
    nt_s = tuple(int(x) for x in nt[perm])
    vp_s = vp[perm]
    k_off = np.zeros(B + 1, np.int64)
    k_off[1:] = np.cumsum(vp_s)
    TOTK = int(k_off[-1])
    maxpad = int(vp_s[0])

    bt = np.asarray(block_tables, np.int64)
    ident_bt = np.arange(B * PAGES, dtype=np.int64).reshape(B, PAGES)
    identity_layout = bt.shape == (B, PAGES) and np.array_equal(bt, ident_bt)
    slot_mapping = np.asarray(slot_mapping, np.int64)

    # mask [128, maxpad] in block-row layout; shared across cores
    mask = np.zeros((P, maxpad), np.float32)
    for s in range(B):
        b = perm[s]
        rb = _rowbase(s)
        mask[rb:rb + 4, :min(int(ctx[b]), maxpad)] = 1.0

    in_maps = []
    for m in range(NCORES):
        qT = np.zeros((P, B * 32), np.float32)
        kT_packed = np.empty((P, TOTK), np.float32)
        vt_packed = np.empty((P, TOTK), np.float32)
        kc = k_cache[:, m, :]
        vc = v_cache[:, m, :]
        for s in range(B):
            b = int(perm[s])
            vps = int(vp_s[s])
            nts = int(nt_s[s])
            o0 = int(k_off[s])
            col = 32 * s + 4 * (s % 8)
            qT[:, col:col + 4] = q[b, 4 * m:4 * m + 4, :].T * SCALE

            if identity_layout:
                sids = None
                krows = kc[b * KV_LEN:b * KV_LEN + vps]
                vrows = vc[b * KV_LEN:b * KV_LEN + vps]
            else:
                sids = (bt[b, :, None] * PAGE
                        + np.arange(PAGE)[None, :]).reshape(-1)[:vps]
                krows = kc[sids]
                vrows = vc[sids]

            # scatter of the new tokens (store_kvcache semantics)
            patch = None
            if identity_layout:
                rel = slot_mapping - b * KV_LEN
                hit = np.nonzero((rel >= 0) & (rel < vps))[0]
                if hit.size:
                    patch = (rel[hit], hit)
            else:
                eq = sids[:, None] == slot_mapping[None, :]
                pos, src = np.nonzero(eq)
                if pos.size:
                    patch = (pos, src)
            if patch is not None:
                krows = krows.copy()
                vrows = vrows.copy()
                krows[patch[0]] = k[patch[1], m, :]
                vrows[patch[0]] = v[patch[1], m, :]

            kT_packed[:, o0:o0 + vps] = krows.T
            vt_packed[:, o0:o0 + vps] = (
                vrows.reshape(nts, CH, D).transpose(1, 0, 2).reshape(CH, -1))

        in_maps.append(dict(qT=qT, kT=kT_packed, vt=vt_packed, mask=mask))

    return in_maps, perm, nt_s


def kernel(q, k, v, k_cache, v_cache, slot_mapping, block_tables,
           context_lens, _trace=False):
    from concourse import bass_utils

    q = np.asarray(q, np.float32)
    k = np.asarray(k, np.float32)
    v = np.asarray(v, np.float32)
    k_cache = np.asarray(k_cache, np.float32)
    v_cache = np.asarray(v_cache, np.float32)

    in_maps, perm, nt_s = _host_prep(
        q, k, v, k_cache, v_cache, slot_mapping, block_tables, context_lens)

    if nt_s not in _PROGRAM_CACHE:
        _PROGRAM_CACHE[nt_s] = _build_program(nt_s)
    nc = _PROGRAM_CACHE[nt_s]

    res = bass_utils.run_bass_kernel_spmd(
        nc, in_maps, core_ids=list(range(NCORES)), trace=_trace)

    o = np.empty((B, H_Q, D), np.float32)
    for m in range(NCORES):
        oT = res.results[m]["oT"]
        r = res.results[m]["r"][:, 0]
        for s in range(B):
            b = int(perm[s])
            rb = _rowbase(s)
            o[b, 4 * m:4 * m + 4, :] = (oT[:, rb:rb + 4] * r[rb:rb + 4]).T
    if _trace:
        kernel._last_result = res
    return o


# revision 20
# speedup vs baseline: 1.4318x; 1.4318x over previous
"""Paged-attention GQA decode kernel for Trainium2 (8 NeuronCores).

Problem: B=32 sequences, one new token each; KV cache [65536, 8, 128] f32
paged with PAGE=16; 32 query heads, 8 KV heads (GQA group 4), D=128.

Sharding: each core owns 2 KV heads x 16 sequences (batch split in two
interleaved-by-length halves for balance). Per core there are 32 "units"
(seq, head), each contributing 4 query rows -> 128 partition rows.

Device pipeline per core:
  mm1:  scores[4u+g, j] = q_u . K_u[j]  -- block-diagonal accumulation of
        M=128 matmuls (f32r, N=512) into one scores PSUM tile, valid
        (128-padded, length-sorted) context only.
  softmax: exp (ACT) -> masked multiply + row-sum in one DVE pass
        (tensor_tensor_reduce); normalization deferred to host via r=1/sum.
  pT:   PE transposes of the masked-exp tile (chunks of 128 positions).
  mm2:  o_u += pT_chunk(4 cols, stationary) @ V_pair_chunk[128, 256]
        (f32r N=256 moving operand, both heads' V interleaved; the
        irrelevant head half is discarded on the PSUM->SBUF copy).
  host: o = o_rows * r, un-permute.

The program is value-specialized on the sorted per-unit chunk counts
(from context_lens); compiled programs are cached per signature.
"""

import numpy as np

B = 32
KV_LEN = 2048
PAGE = 16
PAGES = KV_LEN // PAGE
H_Q = 32
H_KV = 8
D = 128
CH = 128                 # slot chunk (matmul contraction tile)
NCORES = 8
P = 128
SPC = 16                 # seqs per core
UPC = 32                 # units (seq, head) per core
SCALE = np.float32(1.0 / np.sqrt(D))

_PROGRAM_CACHE = {}
_USE_TTR = False   # tensor_tensor_reduce (custom DVE op) on/off


def _k_groups(vps, max_cols):
    """Greedy-pack consecutive units into DMA groups of <=max_cols."""
    groups, cur, cols = [], [], 0
    for i, c in enumerate(vps):
        if cur and cols + c > max_cols:
            groups.append(cur)
            cur, cols = [], 0
        cur.append(i)
        cols += c
    if cur:
        groups.append(cur)
    return groups


def _build_program(ntU):
    """Build + compile the per-core program. ntU = per-seq-slot chunk counts
    (descending, len 16); unit u=2s+h has ntU[s] chunks."""
    import concourse.bacc as bacc
    import concourse.mybir as mybir
    import concourse.tile as tile
    from concourse.masks import make_identity
    from concourse.tile import add_dep_helper

    f32 = mybir.dt.float32
    f32r = mybir.dt.float32r

    vpU = [ntU[u // 2] * CH for u in range(UPC)]          # per-unit K cols
    k_off = np.zeros(UPC + 1, np.int64)
    k_off[1:] = np.cumsum(vpU)
    TOTK = int(k_off[-1])
    v_cols = [ntU[s] * 2 * CH for s in range(SPC)]         # per-seq V cols
    v_off = np.zeros(SPC + 1, np.int64)
    v_off[1:] = np.cumsum(v_cols)
    assert int(v_off[-1]) == TOTK
    maxpad = vpU[0]
    maxnt = ntU[0]

    kgroups = _k_groups(vpU, 2048)
    vgroups = _k_groups(v_cols, 4096)

    nc = bacc.Bacc(
        "TRN2",
        target_bir_lowering=False,
        debug=False,
        enable_asserts=False,
        num_devices=NCORES,
    )
    qT_d = nc.dram_tensor("qT", [P, UPC * P], f32r, kind="ExternalInput").ap()
    kT_d = nc.dram_tensor("kT", [P, TOTK], f32r, kind="ExternalInput").ap()
    vt_d = nc.dram_tensor("vt", [P, TOTK], f32r, kind="ExternalInput").ap()
    mask_d = nc.dram_tensor("mask", [P, maxpad], f32, kind="ExternalInput").ap()
    o_d = nc.dram_tensor("o", [P, P], f32, kind="ExternalOutput").ap()
    r_d = nc.dram_tensor("r", [P, 1], f32, kind="ExternalOutput").ap()

    with tile.TileContext(nc) as tc:
        with (
            tc.tile_pool(name="const", bufs=1) as constp,
            tc.tile_pool(name="kpool", bufs=6) as kpool,
            tc.tile_pool(name="vpool", bufs=3) as vpool,
            tc.tile_pool(name="sm", bufs=1) as smp,
            tc.tile_pool(name="tp", bufs=2, space="PSUM") as tpp,
            tc.tile_pool(name="ps_scores", bufs=1, space="PSUM") as pssc,
            tc.tile_pool(name="ps_o", bufs=2, space="PSUM") as pso,
        ):
            # constants ride the scalar HWDGE queue; K stays on sync so the
            # first K group lands (and mm1 starts) as early as possible.
            qT_s = constp.tile([P, UPC * P], f32r)
            nc.scalar.dma_start(qT_s[:], qT_d[:])
            mask_s = constp.tile([P, maxpad], f32)
            nc.scalar.dma_start(mask_s[:], mask_d[:])
            ident = constp.tile([P, P], f32)
            make_identity(nc, ident[:])

            e_t = smp.tile([P, maxpad], f32)
            scores_ps = pssc.tile([P, maxpad], f32, space="PSUM")

            # ---- phase 1: grouped K loads + block-diagonal mm1 ----
            # Unit u's lhsT slice has its q columns at out rows 4u..4u+4,
            # zeros elsewhere; all units accumulate into the shared scores
            # PSUM via pending-zero semantics (unit 0 opens every bank).
            first_mm = {}
            for grp in kgroups:
                g0, g1 = int(k_off[grp[0]]), int(k_off[grp[-1] + 1])
                kt = kpool.tile([P, 2048], f32r, tag="k")
                nc.sync.dma_start(kt[:, :g1 - g0], kT_d[:, g0:g1])
                for u in grp:
                    vps = vpU[u]
                    uoff = int(k_off[u]) - g0
                    for r0 in range(0, vps, 512):
                        n = min(512, vps - r0)
                        bank = r0 // 512
                        mm = nc.tensor.matmul(
                            scores_ps[:, r0:r0 + n],
                            qT_s[:, P * u:P * (u + 1)],
                            kt[:, uoff + r0:uoff + r0 + n],
                            start=(u == 0), stop=False,
                            skip_group_check=True,
                        )
                        if u == 0:
                            first_mm[bank] = mm
                        else:
                            add_dep_helper(
                                mm.ins, first_mm[bank].ins,
                                reason="pending-zero: unit-0 mm1 first")

            # ---- softmax (deferred normalization) ----
            nc.scalar.activation(e_t[:], scores_ps[:],
                                 mybir.ActivationFunctionType.Exp)
            em_t = smp.tile([P, maxpad], f32)
            s_t = smp.tile([P, 1], f32)
            if _USE_TTR:
                nc.vector.tensor_tensor_reduce(
                    out=em_t[:], in0=e_t[:], in1=mask_s[:],
                    scale=1.0, scalar=0.0,
                    op0=mybir.AluOpType.mult, op1=mybir.AluOpType.add,
                    accum_out=s_t[:])
            else:
                nc.vector.tensor_mul(em_t[:], e_t[:], mask_s[:])
                nc.vector.reduce_sum(s_t[:], em_t[:], axis=mybir.AxisListType.X)
            r_t = smp.tile([P, 1], f32)
            nc.vector.reciprocal(r_t[:], s_t[:])
            nc.sync.dma_start(r_d[:], r_t[:])

            # ---- transposes of masked-exp chunks ----
            pT = smp.tile([P, maxnt * CH], f32r)
            for c in range(maxnt):
                tp = tpp.tile([P, P], f32, space="PSUM", tag="tp")
                nc.tensor.transpose(tp[:], em_t[:, c * CH:(c + 1) * CH], ident[:])
                nc.vector.tensor_copy(pT[:, c * CH:(c + 1) * CH], tp[:])

            # ---- phase 2: grouped V loads + p-stationary mm2 ----
            # moving operand = [V_h0 | V_h1] chunk [128, 256] (f32r, N=256);
            # each unit keeps only its head's half of the [4, 256] result.
            o_sb = smp.tile([4, UPC * D], f32)
            for grp in vgroups:
                g0, g1 = int(v_off[grp[0]]), int(v_off[grp[-1] + 1])
                vt = vpool.tile([P, 4096], f32r, tag="v")
                nc.scalar.dma_start(vt[:, :g1 - g0], vt_d[:, g0:g1])
                for s in grp:
                    soff = int(v_off[s]) - g0
                    for h in (0, 1):
                        u = 2 * s + h
                        o_ps = pso.tile([4, 2 * D], f32, space="PSUM", tag="o")
                        for c in range(ntU[s]):
                            nc.tensor.matmul(
                                o_ps[:, :],
                                pT[:, c * CH + 4 * u:c * CH + 4 * u + 4],
                                vt[:, soff + 2 * c * D:soff + 2 * (c + 1) * D],
                                start=(c == 0), stop=(c == ntU[s] - 1),
                            )
                        nc.vector.tensor_copy(
                            o_sb[:, D * u:D * (u + 1)],
                            o_ps[:, D * h:D * (h + 1)])

            nc.sync.dma_start(
                o_d.rearrange("(u g) d -> g u d", g=4),
                o_sb[:].rearrange("g (u d) -> g u d", u=UPC))

    nc.compile()
    return nc


def _host_prep(q, k, v, k_cache, v_cache, slot_mapping, block_tables,
               context_lens):
    """Build per-core packed inputs. Returns (in_maps, perm, ntU)."""
    ctx = np.clip(np.asarray(context_lens, np.int64), 1, KV_LEN)
    nt = (ctx + CH - 1) // CH
    perm = np.argsort(-nt, kind="stable")        # global length-sorted order
    nt_sorted = nt[perm]
    ntU = tuple(int(x) for x in nt_sorted[0::2])  # per-slot padded chunk count

    vpU = [ntU[u // 2] * CH for u in range(UPC)]
    k_off = np.zeros(UPC + 1, np.int64)
    k_off[1:] = np.cumsum(vpU)
    TOTK = int(k_off[-1])
    v_cols = [ntU[s] * 2 * CH for s in range(SPC)]
    v_off = np.zeros(SPC + 1, np.int64)
    v_off[1:] = np.cumsum(v_cols)
    maxpad = vpU[0]

    bt = np.asarray(block_tables, np.int64)
    ident_bt = np.arange(B * PAGES, dtype=np.int64).reshape(B, PAGES)
    identity_layout = bt.shape == (B, PAGES) and np.array_equal(bt, ident_bt)
    slot_mapping = np.asarray(slot_mapping, np.int64)

    def gather_rows(cache_h, b, vpa):
        """rows [0, vpa) of seq b's context for one head slice, with the
        new-token scatter applied."""
        if identity_layout:
            rows = cache_h[b * KV_LEN:b * KV_LEN + vpa]
            rel = slot_mapping - b * KV_LEN
            hit = np.nonzero((rel >= 0) & (rel < vpa))[0]
            patch = (rel[hit], hit) if hit.size else None
        else:
            sids = (bt[b, :, None] * PAGE
                    + np.arange(PAGE)[None, :]).reshape(-1)[:vpa]
            rows = cache_h[sids]
            pos, src = np.nonzero(sids[:, None] == slot_mapping[None, :])
            patch = (pos, src) if pos.size else None
        return rows, patch

    in_maps = []
    for m in range(NCORES):
        hb, hp = m // 4, m % 4
        qT = np.zeros((P, UPC * P), np.float32)
        kT_packed = np.zeros((P, TOTK), np.float32)
        vt_packed = np.zeros((P, TOTK), np.float32)
        mask = np.zeros((P, maxpad), np.float32)
        for s in range(SPC):
            b = int(perm[2 * s + hb])
            ntu = ntU[s]
            vpa = min(int(nt[b]), ntu) * CH   # actual cols (<= padded)
            for h in (0, 1):
                u = 2 * s + h
                head = 2 * hp + h
                col = P * u + 4 * u
                qT[:, col:col + 4] = q[b, 4 * head:4 * head + 4, :].T * SCALE
                mask[4 * u:4 * u + 4, :min(int(ctx[b]), maxpad)] = 1.0

                krows, patch = gather_rows(k_cache[:, head, :], b, vpa)
                if patch is not None:
                    krows = krows.copy()
                    krows[patch[0]] = k[patch[1], head, :]
                o0 = int(k_off[u])
                kT_packed[:, o0:o0 + vpa] = krows.T

                vrows, patch = gather_rows(v_cache[:, head, :], b, vpa)
                if patch is not None:
                    vrows = vrows.copy()
                    vrows[patch[0]] = v[patch[1], head, :]
                # vt layout per seq: [jj, c, h, d]
                vo = int(v_off[s])
                vt3 = vt_packed[:, vo:vo + ntu * 2 * D].reshape(P, ntu, 2, D)
                nc_ = vpa // CH
                vt3[:, :nc_, h, :] = (
                    vrows.reshape(nc_, CH, D).transpose(1, 0, 2))

        in_maps.append(dict(qT=qT, kT=kT_packed, vt=vt_packed, mask=mask))

    return in_maps, perm, ntU


def kernel(q, k, v, k_cache, v_cache, slot_mapping, block_tables,
           context_lens, _trace=False):
    from concourse import bass_utils

    q = np.asarray(q, np.float32)
    k = np.asarray(k, np.float32)
    v = np.asarray(v, np.float32)
    k_cache = np.asarray(k_cache, np.float32)
    v_cache = np.asarray(v_cache, np.float32)

    in_maps, perm, ntU = _host_prep(
        q, k, v, k_cache, v_cache, slot_mapping, block_tables, context_lens)

    if ntU not in _PROGRAM_CACHE:
        _PROGRAM_CACHE[ntU] = _build_program(ntU)
    nc = _PROGRAM_CACHE[ntU]

    res = bass_utils.run_bass_kernel_spmd(
        nc, in_maps, core_ids=list(range(NCORES)), trace=_trace)

    o = np.empty((B, H_Q, D), np.float32)
    for m in range(NCORES):
        hb, hp = m // 4, m % 4
        om = res.results[m]["o"]
        r = res.results[m]["r"][:, 0]
        for s in range(SPC):
            b = int(perm[2 * s + hb])
            for h in (0, 1):
                u = 2 * s + h
                head = 2 * hp + h
                o[b, 4 * head:4 * head + 4, :] = (
                    om[4 * u:4 * u + 4, :] * r[4 * u:4 * u + 4, None])
    if _trace:
        kernel._last_result = res
    return o


# revision 22
# speedup vs baseline: 1.4604x; 1.0200x over previous
"""Paged-attention GQA decode kernel for Trainium2 (8 NeuronCores).

Problem: B=32 sequences, one new token each; KV cache [65536, 8, 128] f32
paged with PAGE=16; 32 query heads, 8 KV heads (GQA group 4), D=128.

Sharding: each core owns 2 KV heads x 16 sequences (batch split in two
interleaved-by-length halves for balance). Per core there are 32 "units"
(seq, head), each contributing 4 query rows -> 128 partition rows.

Device pipeline per core:
  mm1:  scores[4u+g, j] = q_u . K_u[j]  -- block-diagonal accumulation of
        M=128 matmuls (f32r, N=512) into one scores PSUM tile, valid
        (128-padded, length-sorted) context only.
  softmax: exp (ACT) -> masked multiply + row-sum in one DVE pass
        (tensor_tensor_reduce); normalization deferred to host via r=1/sum.
  pT:   PE transposes of the masked-exp tile (chunks of 128 positions).
  mm2:  o_u += pT_chunk(4 cols, stationary) @ V_pair_chunk[128, 256]
        (f32r N=256 moving operand, both heads' V interleaved; the
        irrelevant head half is discarded on the PSUM->SBUF copy).
  host: o = o_rows * r, un-permute.

The program is value-specialized on the sorted per-unit chunk counts
(from context_lens); compiled programs are cached per signature.
"""

import numpy as np

B = 32
KV_LEN = 2048
PAGE = 16
PAGES = KV_LEN // PAGE
H_Q = 32
H_KV = 8
D = 128
CH = 128                 # slot chunk (matmul contraction tile)
NCORES = 8
P = 128
SPC = 16                 # seqs per core
UPC = 32                 # units (seq, head) per core
SCALE = np.float32(1.0 / np.sqrt(D))

_PROGRAM_CACHE = {}
_USE_TTR = False   # tensor_tensor_reduce (custom DVE op) on/off


def _k_groups(vps, max_cols):
    """Greedy-pack consecutive units into DMA groups of <=max_cols."""
    groups, cur, cols = [], [], 0
    for i, c in enumerate(vps):
        if cur and cols + c > max_cols:
            groups.append(cur)
            cur, cols = [], 0
        cur.append(i)
        cols += c
    if cur:
        groups.append(cur)
    return groups


def _build_program(ntU):
    """Build + compile the per-core program. ntU = per-seq-slot chunk counts
    (descending, len 16); unit u=2s+h has ntU[s] chunks."""
    import concourse.bacc as bacc
    import concourse.mybir as mybir
    import concourse.tile as tile
    from concourse.masks import make_identity
    from concourse.tile import add_dep_helper

    f32 = mybir.dt.float32
    f32r = mybir.dt.float32r

    vpU = [ntU[u // 2] * CH for u in range(UPC)]          # per-unit K cols
    k_off = np.zeros(UPC + 1, np.int64)
    k_off[1:] = np.cumsum(vpU)
    TOTK = int(k_off[-1])
    v_cols = [ntU[s] * 2 * CH for s in range(SPC)]         # per-seq V cols
    v_off = np.zeros(SPC + 1, np.int64)
    v_off[1:] = np.cumsum(v_cols)
    assert int(v_off[-1]) == TOTK
    maxpad = vpU[0]
    maxnt = ntU[0]

    kgroups = _k_groups(vpU, 2048)
    vgroups = _k_groups(v_cols, 4096)

    nc = bacc.Bacc(
        "TRN2",
        target_bir_lowering=False,
        debug=False,
        enable_asserts=False,
        num_devices=NCORES,
    )
    qT_d = nc.dram_tensor("qT", [P, UPC * P], f32r, kind="ExternalInput").ap()
    kT_d = nc.dram_tensor("kT", [P, TOTK], f32r, kind="ExternalInput").ap()
    vt_d = nc.dram_tensor("vt", [P, TOTK], f32r, kind="ExternalInput").ap()
    mask_d = nc.dram_tensor("mask", [P, maxpad], f32, kind="ExternalInput").ap()
    o_d = nc.dram_tensor("o", [P, P], f32, kind="ExternalOutput").ap()
    r_d = nc.dram_tensor("r", [P, 1], f32, kind="ExternalOutput").ap()

    with tile.TileContext(nc) as tc:
        with (
            tc.tile_pool(name="const", bufs=1) as constp,
            tc.tile_pool(name="kpool", bufs=6) as kpool,
            tc.tile_pool(name="vpool", bufs=5) as vpool,
            tc.tile_pool(name="sm", bufs=1) as smp,
            tc.tile_pool(name="tp", bufs=2, space="PSUM") as tpp,
            tc.tile_pool(name="ps_scores", bufs=1, space="PSUM") as pssc,
            tc.tile_pool(name="ps_o", bufs=2, space="PSUM") as pso,
        ):
            # constants ride the scalar HWDGE queue; K stays on sync so the
            # first K group lands (and mm1 starts) as early as possible.
            qT_s = constp.tile([P, UPC * P], f32r)
            nc.scalar.dma_start(qT_s[:, :8 * P], qT_d[:, :8 * P])
            nc.scalar.dma_start(qT_s[:, 8 * P:], qT_d[:, 8 * P:])
            mask_s = constp.tile([P, maxpad], f32)
            nc.scalar.dma_start(mask_s[:], mask_d[:])
            ident = constp.tile([P, P], f32)
            make_identity(nc, ident[:])

            e_t = smp.tile([P, maxpad], f32)
            scores_ps = pssc.tile([P, maxpad], f32, space="PSUM")

            # ---- phase 1: grouped K loads + block-diagonal mm1 ----
            # Unit u's lhsT slice has its q columns at out rows 4u..4u+4,
            # zeros elsewhere; all units accumulate into the shared scores
            # PSUM via pending-zero semantics (unit 0 opens every bank).
            first_mm = {}
            for grp in kgroups:
                g0, g1 = int(k_off[grp[0]]), int(k_off[grp[-1] + 1])
                kt = kpool.tile([P, 2048], f32r, tag="k")
                nc.sync.dma_start(kt[:, :g1 - g0], kT_d[:, g0:g1])
                for u in grp:
                    vps = vpU[u]
                    uoff = int(k_off[u]) - g0
                    for r0 in range(0, vps, 512):
                        n = min(512, vps - r0)
                        bank = r0 // 512
                        mm = nc.tensor.matmul(
                            scores_ps[:, r0:r0 + n],
                            qT_s[:, P * u:P * (u + 1)],
                            kt[:, uoff + r0:uoff + r0 + n],
                            start=(u == 0), stop=False,
                            skip_group_check=True,
                        )
                        if u == 0:
                            first_mm[bank] = mm
                        else:
                            add_dep_helper(
                                mm.ins, first_mm[bank].ins,
                                reason="pending-zero: unit-0 mm1 first")

            # ---- softmax (deferred normalization) ----
            nc.scalar.activation(e_t[:], scores_ps[:],
                                 mybir.ActivationFunctionType.Exp)
            em_t = smp.tile([P, maxpad], f32)
            s_t = smp.tile([P, 1], f32)
            if _USE_TTR:
                nc.vector.tensor_tensor_reduce(
                    out=em_t[:], in0=e_t[:], in1=mask_s[:],
                    scale=1.0, scalar=0.0,
                    op0=mybir.AluOpType.mult, op1=mybir.AluOpType.add,
                    accum_out=s_t[:])
            else:
                nc.vector.tensor_mul(em_t[:], e_t[:], mask_s[:])
                nc.vector.reduce_sum(s_t[:], em_t[:], axis=mybir.AxisListType.X)
            r_t = smp.tile([P, 1], f32)
            nc.vector.reciprocal(r_t[:], s_t[:])
            nc.sync.dma_start(r_d[:], r_t[:])

            # ---- transposes of masked-exp chunks ----
            pT = smp.tile([P, maxnt * CH], f32r)
            for c in range(maxnt):
                tp = tpp.tile([P, P], f32, space="PSUM", tag="tp")
                nc.tensor.transpose(tp[:], em_t[:, c * CH:(c + 1) * CH], ident[:])
                nc.vector.tensor_copy(pT[:, c * CH:(c + 1) * CH], tp[:])

            # ---- phase 2: grouped V loads + p-stationary mm2 ----
            # moving operand = [V_h0 | V_h1] chunk [128, 256] (f32r, N=256);
            # each unit keeps only its head's half of the [4, 256] result.
            o_sb = smp.tile([4, UPC * D], f32)
            for grp in vgroups:
                g0, g1 = int(v_off[grp[0]]), int(v_off[grp[-1] + 1])
                vt = vpool.tile([P, 4096], f32r, tag="v")
                nc.scalar.dma_start(vt[:, :g1 - g0], vt_d[:, g0:g1])
                for s in grp:
                    soff = int(v_off[s]) - g0
                    for h in (0, 1):
                        u = 2 * s + h
                        o_ps = pso.tile([4, 2 * D], f32, space="PSUM", tag="o")
                        for c in range(ntU[s]):
                            nc.tensor.matmul(
                                o_ps[:, :],
                                pT[:, c * CH + 4 * u:c * CH + 4 * u + 4],
                                vt[:, soff + 2 * c * D:soff + 2 * (c + 1) * D],
                                start=(c == 0), stop=(c == ntU[s] - 1),
                            )
                        nc.vector.tensor_copy(
                            o_sb[:, D * u:D * (u + 1)],
                            o_ps[:, D * h:D * (h + 1)])

            nc.sync.dma_start(
                o_d.rearrange("(u g) d -> g u d", g=4),
                o_sb[:].rearrange("g (u d) -> g u d", u=UPC))

    nc.compile()
    return nc


def _host_prep(q, k, v, k_cache, v_cache, slot_mapping, block_tables,
               context_lens):
    """Build per-core packed inputs. Returns (in_maps, perm, ntU)."""
    ctx = np.clip(np.asarray(context_lens, np.int64), 1, KV_LEN)
    nt = (ctx + CH - 1) // CH
    perm = np.argsort(-nt, kind="stable")        # global length-sorted order
    nt_sorted = nt[perm]
    ntU = tuple(int(x) for x in nt_sorted[0::2])  # per-slot padded chunk count

    vpU = [ntU[u // 2] * CH for u in range(UPC)]
    k_off = np.zeros(UPC + 1, np.int64)
    k_off[1:] = np.cumsum(vpU)
    TOTK = int(k_off[-1])
    v_cols = [ntU[s] * 2 * CH for s in range(SPC)]
    v_off = np.zeros(SPC + 1, np.int64)
    v_off[1:] = np.cumsum(v_cols)
    maxpad = vpU[0]

    bt = np.asarray(block_tables, np.int64)
    ident_bt = np.arange(B * PAGES, dtype=np.int64).reshape(B, PAGES)
    identity_layout = bt.shape == (B, PAGES) and np.array_equal(bt, ident_bt)
    slot_mapping = np.asarray(slot_mapping, np.int64)

    def gather_rows(cache_h, b, vpa):
        """rows [0, vpa) of seq b's context for one head slice, with the
        new-token scatter applied."""
        if identity_layout:
            rows = cache_h[b * KV_LEN:b * KV_LEN + vpa]
            rel = slot_mapping - b * KV_LEN
            hit = np.nonzero((rel >= 0) & (rel < vpa))[0]
            patch = (rel[hit], hit) if hit.size else None
        else:
            sids = (bt[b, :, None] * PAGE
                    + np.arange(PAGE)[None, :]).reshape(-1)[:vpa]
            rows = cache_h[sids]
            pos, src = np.nonzero(sids[:, None] == slot_mapping[None, :])
            patch = (pos, src) if pos.size else None
        return rows, patch

    in_maps = []
    for m in range(NCORES):
        hb, hp = m // 4, m % 4
        qT = np.zeros((P, UPC * P), np.float32)
        kT_packed = np.zeros((P, TOTK), np.float32)
        vt_packed = np.zeros((P, TOTK), np.float32)
        mask = np.zeros((P, maxpad), np.float32)
        for s in range(SPC):
            b = int(perm[2 * s + hb])
            ntu = ntU[s]
            vpa = min(int(nt[b]), ntu) * CH   # actual cols (<= padded)
            for h in (0, 1):
                u = 2 * s + h
                head = 2 * hp + h
                col = P * u + 4 * u
                qT[:, col:col + 4] = q[b, 4 * head:4 * head + 4, :].T * SCALE
                mask[4 * u:4 * u + 4, :min(int(ctx[b]), maxpad)] = 1.0

                krows, patch = gather_rows(k_cache[:, head, :], b, vpa)
                if patch is not None:
                    krows = krows.copy()
                    krows[patch[0]] = k[patch[1], head, :]
                o0 = int(k_off[u])
                kT_packed[:, o0:o0 + vpa] = krows.T

                vrows, patch = gather_rows(v_cache[:, head, :], b, vpa)
                if patch is not None:
                    vrows = vrows.copy()
                    vrows[patch[0]] = v[patch[1], head, :]
                # vt layout per seq: [jj, c, h, d]
                vo = int(v_off[s])
                vt3 = vt_packed[:, vo:vo + ntu * 2 * D].reshape(P, ntu, 2, D)
                nc_ = vpa // CH
                vt3[:, :nc_, h, :] = (
                    vrows.reshape(nc_, CH, D).transpose(1, 0, 2))

        in_maps.append(dict(qT=qT, kT=kT_packed, vt=vt_packed, mask=mask))

    return in_maps, perm, ntU


def kernel(q, k, v, k_cache, v_cache, slot_mapping, block_tables,
           context_lens, _trace=False):
    from concourse import bass_utils

    q = np.asarray(q, np.float32)
    k = np.asarray(k, np.float32)
    v = np.asarray(v, np.float32)
    k_cache = np.asarray(k_cache, np.float32)
    v_cache = np.asarray(v_cache, np.float32)

    in_maps, perm, ntU = _host_prep(
        q, k, v, k_cache, v_cache, slot_mapping, block_tables, context_lens)

    if ntU not in _PROGRAM_CACHE:
        _PROGRAM_CACHE[ntU] = _build_program(ntU)
    nc = _PROGRAM_CACHE[ntU]

    res = bass_utils.run_bass_kernel_spmd(
        nc, in_maps, core_ids=list(range(NCORES)), trace=_trace)

    o = np.empty((B, H_Q, D), np.float32)
    for m in range(NCORES):
        hb, hp = m // 4, m % 4
        om = res.results[m]["o"]
        r = res.results[m]["r"][:, 0]
        for s in range(SPC):
            b = int(perm[2 * s + hb])
            for h in (0, 1):
                u = 2 * s + h
                head = 2 * hp + h
                o[b, 4 * head:4 * head + 4, :] = (
                    om[4 * u:4 * u + 4, :] * r[4 * u:4 * u + 4, None])
    if _trace:
        kernel._last_result = res
    return o
